# revision 3
# baseline (speedup 1.0000x reference)
"""Two-layer GCN on 8 NeuronCores (Trainium2, Bass/Tile).

Math (PyG GCNConv order, matching the reference):
    A = D^-1/2 (Adj + I) D^-1/2          (deg over dst, incl. self loops)
    h1 = relu(A @ (x @ W1) + b1)
    out = log_softmax(A @ (h1 @ W2) + b2)

Restructuring used here:
  *  A @ (h1 @ W2) == (A @ h1) @ W2  -- both sparse aggregations run on
     16-wide rows; the 16->64 dense expansion happens after aggregation.
  *  A's normalization is separable: pre-scale table rows by dinv[src],
     post-scale aggregated rows by dinv[dst]; the self loop becomes one
     extra ELL slot pointing at the node's own (pre-scaled) row.
  *  Nodes are sharded across the 8 cores.  Each core aggregates its
     12500 dst nodes from a replicated 16-wide table.  Dst nodes are
     degree-sorted so each 128-row ELL tile pads only to its own max
     degree; the tile profile is shared by all cores (max over cores) so
     one program serves all 8 cores SPMD.
  *  The ELL index array is preloaded in ONE direct DMA, and the row
     gathers are batched into a few large indirect DMAs (chunks of many
     tiles) so the SWDGE descriptor-generation fixed cost (~1us per
     instruction) is amortized and descriptor generation pipelines with
     the SDMA transfers.

Device work is 3 SPMD launches: (A) t1'' = dinv*(x@W1) per shard,
(B) h1'' = dinv*relu(dinv*agg(t1'') + b1), (C) out = log_softmax(
(dinv*agg(h1'')) @ W2 + b2).  The host only reorders integer index
arrays and concatenates shard outputs between launches.
"""

import numpy as np

N_NODES = 100000
N_CORES = 8
PER = N_NODES // N_CORES  # 12500
P = 128
HID = 16
OUT = 64
IN_CH = 512
N_TILES = (PER + P - 1) // P  # 98
PER_PAD = N_TILES * P  # 12544
CHUNK_TILES = 14  # ELL tiles gathered per indirect DMA

LAST_HW_TIMES = []  # exec_time_ns per launch when BASS_TRACE=1


def _log_softmax(h):
    m = h.max(axis=1, keepdims=True)
    e = np.exp(h - m)
    return (h - m) - np.log(e.sum(axis=1, keepdims=True))


def _host_reference_path(x, edge_index, W1, b1, W2, b2):
    src = edge_index[0].astype(np.int64)
    dst = edge_index[1].astype(np.int64)
    deg = (np.bincount(dst, minlength=N_NODES) + 1).astype(np.float32)
    dinv = 1.0 / np.sqrt(deg)

    def agg(h):
        hs = h * dinv[:, None]
        out = np.zeros_like(h)
        np.add.at(out, dst, hs[src])
        out += hs
        return out * dinv[:, None]

    h1 = np.maximum(agg(x @ W1) + b1, 0.0)
    h2 = agg(h1) @ W2 + b2
    return _log_softmax(h2).astype(np.float32)


# ----------------------------------------------------------------------
# graph preprocessing (host, integer work only)
# ----------------------------------------------------------------------

def _build_plan(edge_index):
    """Degree-sorted ELL layout, chunked for batched gathers.

    Returns per-core plans plus the shared tile/chunk profile:
      tile_S[t]   : slots for tile t (max degree over cores in that tile +1)
      chunks      : list of (t0, t1) tile ranges
      ell layout  : [128, total_S] int32 row-major per core; per partition
                    the slot lists of all tiles concatenated in tile order.
    """
    src = np.ascontiguousarray(edge_index[0], dtype=np.int64)
    dst = np.ascontiguousarray(edge_index[1], dtype=np.int64)
    deg = (np.bincount(dst, minlength=N_NODES) + 1).astype(np.float32)
    dinv = (1.0 / np.sqrt(deg)).astype(np.float32)

    order = np.argsort(dst, kind="stable")
    s_sorted = src[order].astype(np.int32)
    d_sorted = dst[order]
    row_ptr = np.searchsorted(d_sorted, np.arange(N_NODES + 1))

    perms = []
    ldegs = []
    for c in range(N_CORES):
        lo, hi = c * PER, (c + 1) * PER
        ldeg = (row_ptr[lo + 1:hi + 1] - row_ptr[lo:hi]).astype(np.int64)
        perms.append(np.argsort(-ldeg, kind="stable"))
        ldegs.append(ldeg)

    # common tile slot-count profile: max over cores per tile position
    tile_S = np.zeros(N_TILES, dtype=np.int64)
    for t in range(N_TILES):
        m = 0
        for c in range(N_CORES):
            nodes = perms[c][t * P:(t + 1) * P]
            if len(nodes):
                m = max(m, int(ldegs[c][nodes].max()))
        tile_S[t] = m + 1  # +1 slot for the self loop

    total_S = int(tile_S.sum())
    tile_off = np.concatenate([[0], np.cumsum(tile_S)]).astype(np.int64)

    plans = []
    for c in range(N_CORES):
        lo = c * PER
        perm = perms[c]
        # ell[p, slot]: per-partition concatenation of all tiles' slots
        ell = np.full((P, total_S), N_NODES, dtype=np.int32)  # pad -> zero row
        for t in range(N_TILES):
            nodes = perm[t * P:(t + 1) * P]
            S = int(tile_S[t])
            o = int(tile_off[t])
            for p, nl in enumerate(nodes):
                g = lo + int(nl)
                e0, e1 = int(row_ptr[g]), int(row_ptr[g + 1])
                k = e1 - e0
                ell[p, o:o + k] = s_sorted[e0:e1]
                ell[p, o + k] = g  # self loop slot
        gperm = lo + perm
        dinv_perm = dinv[gperm].astype(np.float32)
        pad = PER_PAD - PER
        if pad:
            dinv_perm = np.concatenate([dinv_perm, np.zeros(pad, np.float32)])
        plans.append({
            "ell": np.ascontiguousarray(ell.reshape(-1, 1)),
            "perm": perm,
            "dinv_perm": dinv_perm.reshape(-1, 1),
        })
    return plans, dinv, tile_S, tile_off, total_S


# ----------------------------------------------------------------------
# bass kernels
# ----------------------------------------------------------------------

def _neff_linear1():
    """out = dinv_shard * (x_shard @ W1); [PER, 512] -> [PER, 16]."""
    import concourse.bacc as bacc
    import concourse.mybir as mybir
    from concourse import masks
    from concourse.tile import TileContext
    dt = mybir.dt

    nc = bacc.Bacc()
    xs = nc.dram_tensor("xs", (PER, IN_CH), dt.float32, kind="ExternalInput")
    w = nc.dram_tensor("w", (IN_CH, HID), dt.float32, kind="ExternalInput")
    dv = nc.dram_tensor("dv", (PER, 1), dt.float32, kind="ExternalInput")
    out = nc.dram_tensor("out", (PER, HID), dt.float32, kind="ExternalOutput")

    n_full = PER // P  # 97 full tiles
    rem = PER - n_full * P  # 84

    with TileContext(nc) as tc:
        with tc.tile_pool(name="const", bufs=1) as cpool, \
             tc.tile_pool(name="sb", bufs=3) as pool, \
             tc.tile_pool(name="pst", bufs=2, space="PSUM") as psum_t, \
             tc.tile_pool(name="psa", bufs=2, space="PSUM") as psum_a:
            ident = cpool.tile((P, P), dt.bfloat16)
            masks.make_identity(nc, ident[:])
            wt = cpool.tile((P, 4, HID), dt.float32)
            nc.sync.dma_start(wt[:], w[:, :].rearrange("(c p) j -> p c j", c=4))
            wtb = cpool.tile((P, 4, HID), dt.bfloat16)
            nc.vector.tensor_copy(wtb[:], wt[:])
            dvt = cpool.tile((P, n_full + 1), dt.float32)
            nc.sync.dma_start(
                dvt[:, :n_full],
                dv[:n_full * P, :].rearrange("(t p) o -> p (t o)", p=P))
            nc.sync.dma_start(dvt[:rem, n_full:], dv[n_full * P:, :])

            for i in range(n_full + 1):
                rows = P if i < n_full else rem
                xt = pool.tile((P, IN_CH), dt.float32, name=f"xt{i % 3}")
                nc.sync.dma_start(xt[:rows, :], xs[i * P:i * P + rows, :])
                xb = pool.tile((P, IN_CH), dt.bfloat16, name=f"xb{i % 3}")
                nc.scalar.copy(xb[:rows, :], xt[:rows, :])
                xT = pool.tile((P, 4, P), dt.bfloat16, name=f"xT{i % 3}")
                for c in range(4):
                    pt = psum_t.tile((P, P), dt.bfloat16, name=f"pt{(4 * i + c) % 2}")
                    nc.tensor.transpose(pt[:, :rows], xb[:rows, c * P:(c + 1) * P],
                                        ident[:rows, :rows])
                    nc.vector.tensor_copy(xT[:, c, :rows], pt[:, :rows])
                acc = psum_a.tile((P, HID), dt.float32, name=f"acc{i % 2}")
                for c in range(4):
                    nc.tensor.matmul(acc[:rows, :], xT[:, c, :rows], wtb[:, c, :],
                                     start=(c == 0), stop=(c == 3))
                ot = pool.tile((P, HID), dt.float32, name=f"ot{i % 3}")
                nc.vector.tensor_scalar(out=ot[:rows, :], in0=acc[:rows, :],
                                        scalar1=dvt[:rows, i:i + 1], scalar2=None,
                                        op0=mybir.AluOpType.mult)
                nc.sync.dma_start(out[i * P:i * P + rows, :], ot[:rows, :])
    nc.compile()
    return nc


def _neff_agg(tile_S, tile_off, total_S, layer):
    """ELL aggregation of 16-wide rows from a replicated table.

    layer=1: out = dinv * relu(dinv*agg + b1)            [PER_PAD, 16]
    layer=2: out = log_softmax((dinv*agg) @ W2 + b2)     [PER_PAD, 64]
    (agg includes the self loop as an extra ELL slot; table rows are
    pre-scaled by dinv[src].)

    The whole per-core ELL index array [128, total_S] is preloaded in one
    DMA; gathers run as one indirect DMA per chunk of CHUNK_TILES tiles.
    """
    import concourse.bass as bass
    import concourse.bacc as bacc
    import concourse.mybir as mybir
    from concourse import masks
    from concourse.tile import TileContext
    dt = mybir.dt
    AX = mybir.AxisListType

    nc = bacc.Bacc()
    table = nc.dram_tensor("table", (N_NODES + 8, HID), dt.float32,
                           kind="ExternalInput")
    ell = nc.dram_tensor("ell", (P * total_S, 1), dt.int32, kind="ExternalInput")
    dv = nc.dram_tensor("dv", (PER_PAD, 1), dt.float32, kind="ExternalInput")
    fdim = OUT if layer == 2 else HID
    bias = nc.dram_tensor("bias", (P, fdim), dt.float32, kind="ExternalInput")
    if layer == 2:
        w2 = nc.dram_tensor("w2", (HID, OUT), dt.float32, kind="ExternalInput")
    out = nc.dram_tensor("out", (PER_PAD, fdim), dt.float32, kind="ExternalOutput")

    # chunk boundaries in tile index space
    chunks = []
    t0 = 0
    while t0 < N_TILES:
        t1 = min(t0 + CHUNK_TILES, N_TILES)
        chunks.append((t0, t1))
        t0 = t1
    max_chunk_S = max(int(tile_off[t1] - tile_off[t0]) for t0, t1 in chunks)

    with TileContext(nc) as tc:
        with tc.tile_pool(name="const", bufs=1) as cpool, \
             tc.tile_pool(name="g", bufs=2) as gpool, \
             tc.tile_pool(name="sb", bufs=4) as pool, \
             tc.tile_pool(name="ps", bufs=2, space="PSUM") as psum:
            bt = cpool.tile((P, fdim), dt.float32)
            nc.sync.dma_start(bt[:], bias[:, :])
            dvt = cpool.tile((P, N_TILES), dt.float32)
            nc.sync.dma_start(dvt[:], dv[:, :].rearrange("(t p) o -> p (t o)", p=P))
            # whole ELL index array, one DMA
            ixt = cpool.tile((P, total_S), dt.int32)
            nc.sync.dma_start(
                ixt[:], ell[:, :].rearrange("(p s) o -> p (s o)", p=P))
            if layer == 2:
                ident = cpool.tile((P, P), dt.bfloat16)
                masks.make_identity(nc, ident[:])
                w2t = cpool.tile((HID, OUT), dt.float32)
                nc.sync.dma_start(w2t[:], w2[:, :])
                w2b = cpool.tile((HID, OUT), dt.bfloat16)
                nc.vector.tensor_copy(w2b[:], w2t[:])

            for ci, (t0, t1) in enumerate(chunks):
                c_lo = int(tile_off[t0])
                c_S = int(tile_off[t1] - tile_off[t0])
                g = gpool.tile((P, max_chunk_S, HID), dt.float32,
                               name=f"g{ci % 2}", tag="g")
                nc.gpsimd.indirect_dma_start(
                    out=g[:, :c_S, :], out_offset=None, in_=table[:, :],
                    in_offset=bass.IndirectOffsetOnAxis(
                        ap=ixt[:, c_lo:c_lo + c_S], axis=0))
                for t in range(t0, t1):
                    S = int(tile_S[t])
                    o = int(tile_off[t]) - c_lo
                    red = pool.tile((P, HID), dt.float32, name=f"red{t % 4}")
                    if S > 1:
                        nc.vector.tensor_reduce(
                            out=red[:],
                            in_=g[:, o:o + S, :].rearrange("p s f -> p f s"),
                            op=mybir.AluOpType.add, axis=AX.X)
                    else:
                        nc.vector.tensor_copy(red[:], g[:, o, :])
                    if layer == 1:
                        v = pool.tile((P, HID), dt.float32, name=f"v{t % 4}")
                        nc.vector.tensor_scalar(out=v[:], in0=red[:],
                                                scalar1=dvt[:, t:t + 1],
                                                scalar2=None,
                                                op0=mybir.AluOpType.mult)
                        nc.vector.tensor_tensor(out=v[:], in0=v[:], in1=bt[:],
                                                op=mybir.AluOpType.add)
                        nc.vector.tensor_scalar(out=v[:], in0=v[:],
                                                scalar1=0.0, scalar2=None,
                                                op0=mybir.AluOpType.max)
                        nc.vector.tensor_scalar(out=v[:], in0=v[:],
                                                scalar1=dvt[:, t:t + 1],
                                                scalar2=None,
                                                op0=mybir.AluOpType.mult)
                        nc.sync.dma_start(out[t * P:(t + 1) * P, :], v[:])
                    else:
                        vb = pool.tile((P, HID), dt.bfloat16, name=f"vb{t % 4}")
                        nc.vector.tensor_scalar(out=vb[:], in0=red[:],
                                                scalar1=dvt[:, t:t + 1],
                                                scalar2=None,
                                                op0=mybir.AluOpType.mult)
                        vtp = psum.tile((P, P), dt.bfloat16, name=f"vtp{t % 2}")
                        nc.tensor.transpose(vtp[:HID, :], vb[:, :], ident[:])
                        vT = pool.tile((HID, P), dt.bfloat16, name=f"vT{t % 4}")
                        nc.vector.tensor_copy(vT[:], vtp[:HID, :])
                        acc = psum.tile((P, OUT), dt.float32, name=f"acc{t % 2}")
                        nc.tensor.matmul(acc[:], vT[:, :], w2b[:, :],
                                         start=True, stop=True)
                        h2 = pool.tile((P, OUT), dt.float32, name=f"h2{t % 4}")
                        nc.vector.tensor_tensor(out=h2[:], in0=acc[:], in1=bt[:],
                                                op=mybir.AluOpType.add)
                        mx = pool.tile((P, 1), dt.float32, name=f"mx{t % 4}")
                        nc.vector.tensor_reduce(out=mx[:], in_=h2[:],
                                                op=mybir.AluOpType.max, axis=AX.X)
                        xm = pool.tile((P, OUT), dt.float32, name=f"xm{t % 4}")
                        nc.vector.tensor_scalar(out=xm[:], in0=h2[:],
                                                scalar1=mx[:, :1], scalar2=None,
                                                op0=mybir.AluOpType.subtract)
                        ex = pool.tile((P, OUT), dt.float32, name=f"ex{t % 4}")
                        nc.scalar.activation(ex[:], xm[:],
                                             mybir.ActivationFunctionType.Exp)
                        sm = pool.tile((P, 1), dt.float32, name=f"sm{t % 4}")
                        nc.vector.tensor_reduce(out=sm[:], in_=ex[:],
                                                op=mybir.AluOpType.add, axis=AX.X)
                        ls = pool.tile((P, 1), dt.float32, name=f"ls{t % 4}")
                        nc.scalar.activation(ls[:], sm[:],
                                             mybir.ActivationFunctionType.Ln)
                        fo = pool.tile((P, OUT), dt.float32, name=f"fo{t % 4}")
                        nc.vector.tensor_scalar(out=fo[:], in0=xm[:],
                                                scalar1=ls[:, :1], scalar2=None,
                                                op0=mybir.AluOpType.subtract)
                        nc.sync.dma_start(out[t * P:(t + 1) * P, :], fo[:])
    nc.compile()
    return nc


# ----------------------------------------------------------------------
# driver
# ----------------------------------------------------------------------

_NEFF_CACHE = {}


def _run_spmd(nc, in_maps, cores=None):
    import os
    from concourse import bass_utils
    trace = bool(os.environ.get("BASS_TRACE"))
    res = bass_utils.run_bass_kernel_spmd(
        nc, in_maps, cores if cores is not None else list(range(N_CORES)))
    if trace and res.exec_time_ns:
        LAST_HW_TIMES.append(res.exec_time_ns)
        if os.environ.get("BASS_DUMP_TRACE"):
            try:
                i = len(LAST_HW_TIMES)
                print(f"[launch {i}] exec_time_ns={res.exec_time_ns} "
                      f"profile_json={res.profile_json} "
                      f"trace={res.instructions_and_trace[1] if res.instructions_and_trace else None}")
                import pickle
                if res.instructions_and_trace:
                    with open(f"/tmp/gcn_insts_{i}.pkl", "wb") as f:
                        pickle.dump(res.instructions_and_trace[0], f)
            except Exception as e:
                print("trace dump failed:", e)
    return res.results


def _device_path(x, edge_index, W1, b1, W2, b2):
    x = np.ascontiguousarray(x, dtype=np.float32)
    W1 = np.ascontiguousarray(W1, dtype=np.float32)
    W2 = np.ascontiguousarray(W2, dtype=np.float32)
    b1 = np.ascontiguousarray(b1, dtype=np.float32)
    b2 = np.ascontiguousarray(b2, dtype=np.float32)

    plans, dinv, tile_S, tile_off, total_S = _build_plan(edge_index)

    # --- launch A: t1'' = dinv * (x @ W1), node-sharded -----------------
    if "lin1" not in _NEFF_CACHE:
        _NEFF_CACHE["lin1"] = _neff_linear1()
    res = _run_spmd(_NEFF_CACHE["lin1"], [{
        "xs": np.ascontiguousarray(x[c * PER:(c + 1) * PER]),
        "w": W1,
        "dv": np.ascontiguousarray(dinv[c * PER:(c + 1) * PER, None]),
    } for c in range(N_CORES)])
    table1 = np.zeros((N_NODES + 8, HID), dtype=np.float32)
    table1[:N_NODES] = np.concatenate(
        [res[c]["out"] for c in range(N_CORES)], axis=0)

    # --- launch B: layer-1 aggregation ---------------------------------
    key = ("agg", 1, tuple(tile_S))
    if key not in _NEFF_CACHE:
        _NEFF_CACHE[key] = _neff_agg(tile_S, tile_off, total_S, 1)
    b1_rep = np.tile(b1[None, :], (P, 1)).astype(np.float32)
    res = _run_spmd(_NEFF_CACHE[key], [{
        "table": table1,
        "ell": plans[c]["ell"],
        "dv": plans[c]["dinv_perm"],
        "bias": b1_rep,
    } for c in range(N_CORES)])
    table2 = np.zeros((N_NODES + 8, HID), dtype=np.float32)
    for c in range(N_CORES):
        gids = c * PER + plans[c]["perm"]
        table2[gids] = res[c]["out"][:len(gids)]

    # --- launch C: layer-2 aggregation + dense tail ---------------------
    key = ("agg", 2, tuple(tile_S))
    if key not in _NEFF_CACHE:
        _NEFF_CACHE[key] = _neff_agg(tile_S, tile_off, total_S, 2)
    b2_rep = np.tile(b2[None, :], (P, 1)).astype(np.float32)
    res = _run_spmd(_NEFF_CACHE[key], [{
        "table": table2,
        "ell": plans[c]["ell"],
        "dv": plans[c]["dinv_perm"],
        "bias": b2_rep,
        "w2": W2,
    } for c in range(N_CORES)])
    out = np.empty((N_NODES, OUT), dtype=np.float32)
    for c in range(N_CORES):
        gids = c * PER + plans[c]["perm"]
        out[gids] = res[c]["out"][:len(gids)]
    return out


def kernel(x, edge_index, W1, b1, W2, b2):
    import os
    if not os.environ.get("GCN_NO_BASS"):
        try:
            return _device_path(x, edge_index, W1, b1, W2, b2)
        except Exception:
            import traceback
            traceback.print_exc()
    x = np.asarray(x, dtype=np.float32)
    return _host_reference_path(
        x, np.asarray(edge_index), np.asarray(W1, np.float32),
        np.asarray(b1, np.float32), np.asarray(W2, np.float32),
        np.asarray(b2, np.float32))
